# revision 34
# baseline (speedup 1.0000x reference)
"""Trainium2 Bass kernel for nn_DomainDiscriminator.

Network: conv(512->256,k3,s3,p1) -> BN -> conv(256->128,k3,s3,p1) -> BN
         -> reshape -> 12-layer MLP (3200->...->1, no nonlinearities) -> sigmoid.
Input x: [64, 512, 40, 40] f32.  Output: [64, 1] f32.

Strategy (8 NeuronCores):
 - Data-parallel batch shard (8 per core) for the convs.
 - stride==kernel==3 convs are non-overlapping patch matmuls. Conv1 patches are
   built host-side (space-to-depth, free); conv2 patches are read straight out
   of SBUF with strided access patterns (boundary-split matmuls, no im2col).
 - Training-mode BN: conv bias is absorbed exactly by BN; per-channel batch
   stats are exchanged with tiny (1-2KB) AllReduces.
 - The 12 linear layers have no activations between them, so they compose on
   the host (fp64) into a single [3200] vector + scalar bias; the device
   computes a per-channel partial matvec under the BN2 mesh shadow and
   finishes with two tiny matmuls + sigmoid on its own batch shard; the host
   concatenates the 8 per-core output shards.
 - Convs run in bf16 (BN re-normalizes, keeping error ~3e-3).

Latency structure (per core):
 - warmup collective triggered at t~0 straight from an ExternalInput (no
   staging DMA) so the ~60us TOPSP cold start completes before BN1's mesh.
 - startup DMAs fan out over 4 HWDGE rings (tensor/vector/sync/gpsimd) with
   the first x chunk split so the first matmul issues ~5us earlier.
 - BN1 stats are accumulated incrementally per conv1 psum tile on the
   otherwise-idle Vector/ACT engines, so the stats post ~3us after conv1's
   last matmul.
 - conv2 runs as a single 18-matmul psum chain; its stats are read straight
   from PSUM and the weff partial matvec happens under the BN2 mesh.
 - ACT table loads (Sqrt/Sigmoid) are prefetched into idle windows via dummy
   activations so no table load sits on the critical path.
"""

import os
import sys

sys.path.insert(0, "/opt/trn_rl_repo")

import numpy as np

import concourse.bass as bass
import concourse.mybir as mybir
import concourse.tile as tile
from concourse import bacc
from concourse.bass_utils import run_bass_kernel_spmd

F32 = mybir.dt.float32
BF16 = mybir.dt.bfloat16
F8 = mybir.dt.float8e4

NCORES = 8
BL = 8              # batch per core
B = 64              # full batch
EPS = 1e-5

# conv1: [BL,512,40,40] -> [BL,256,14,14]; conv2: -> [BL,128,5,5]
P1 = 196            # 14*14 positions
P2 = 25             # 5*5 positions
NPT = 4             # conv1 psum tiles (2 batch each)
PTW = 2 * P1        # 392 columns per conv1 psum tile

USE_ALLREDUCE = bool(int(os.environ.get("KERNEL_ALLREDUCE", "0")))

_CACHE = {}

KIJ9 = [(ki, kj) for ki in range(3) for kj in range(3)]
# conv2 im2col block offsets within an h1 patch tile [128, 1568]
BLKOFF = {}
_o = 0
for _ki, _kj in KIJ9:
    BLKOFF[(_ki, _kj)] = _o
    _o += (4 if _ki == 0 else 5) * (4 if _kj == 0 else 5) * 8
assert _o == 1568

# conv1 boundary-trimmed im2col: per kij only the valid output rows/cols are
# shipped and matmul'd ((1,1) goes first so psum start= zeroes the full tile)
C1ORDER = [(1, 1), (0, 1), (1, 0), (1, 2), (2, 1), (0, 0), (0, 2), (2, 0), (2, 2)]


def _c1rng(k):
    # valid output index range [lo, hi) for kernel offset k (stride 3, pad 1)
    return (1, 14) if k == 0 else ((0, 13) if k == 2 else (0, 14))


C1OFF = {}
C1W = 0
for _ki, _kj in C1ORDER:
    C1OFF[(_ki, _kj)] = C1W
    (_il, _ih), (_jl, _jh) = _c1rng(_ki), _c1rng(_kj)
    C1W += 2 * (_ih - _il) * (_jh - _jl)
assert C1W == 3200


# ----------------------------------------------------------------------------
# device program
# ----------------------------------------------------------------------------

def _build():
    nc = bacc.Bacc("TRN2", target_bir_lowering=False, debug=False,
                   enable_asserts=True, num_devices=NCORES)

    xprep = nc.dram_tensor("xprep", [4, NPT, 128, C1W], F8,
                           kind="ExternalInput")
    w1p = nc.dram_tensor("w1p", [128, 36, 256], BF16, kind="ExternalInput")
    w2p = nc.dram_tensor("w2p", [128, 18, 128], BF16, kind="ExternalInput")
    weffp = nc.dram_tensor("weffp", [128, 26], F32, kind="ExternalInput")
    bprep = nc.dram_tensor("bprep", [128, 7], F32, kind="ExternalInput")
    out = nc.dram_tensor("out", [BL, 1], F32, kind="ExternalOutput")

    # bprep columns: bn1_g (2), bn1_b (2), bn2_g, bn2_b, beff(row 0)
    BC_BN1G, BC_BN1B, BC_BN2G, BC_BN2B, BC_BEFF = 0, 2, 4, 5, 6
    GROUPS = [list(range(NCORES))]

    with tile.TileContext(nc) as tc:
        with tc.tile_pool(name="wp", bufs=1) as wp, \
             tc.tile_pool(name="xp", bufs=6) as xp, \
             tc.tile_pool(name="hp", bufs=1) as hp, \
             tc.tile_pool(name="sp", bufs=1) as sp, \
             tc.tile_pool(name="cps", bufs=4, space="PSUM") as cps, \
             tc.tile_pool(name="c2p", bufs=1, space="PSUM") as c2p, \
             tc.tile_pool(name="zp", bufs=1, space="PSUM") as zp, \
             tc.tile_pool(name="dram", bufs=1, space="DRAM") as dram:

            # ---------------- collective warm-up ------------------------
            # TOPSP's first collective has a ~60us cold start; trigger it
            # as early as possible (collectives can't read IO tensors, so
            # bounce a tiny input through DRAM scratch first) so the mesh
            # is warm before the BN1 stats exchange (~66us).
            warm_in = dram.tile([1, 4], F32)
            warm_out = dram.tile([NCORES, 1, 4], F32, addr_space="Shared")
            nc.sync.dma_start(warm_in[:], bprep.ap()[0:1, 0:4])
            nc.gpsimd.collective_compute(
                "AllGather", mybir.AluOpType.bypass,
                replica_groups=GROUPS,
                ins=[warm_in.opt()], outs=[warm_out.opt()])

            # ---------------- priority loads ----------------------------
            # Only SP(sync) and Activation(scalar) are fast HWDGE rings;
            # the gpsimd ring is a high-latency SW DGE and only carries
            # late bulk (w2/weff and the last x chunks). DMA completion
            # semaphores fire ~1.5-3us after the data and serialize per
            # ring, so the first-matmul dependencies (w1 cb0 + xt00) are
            # single whole DMAs, one per fast ring.
            w1sb = wp.tile([128, 36 * 256], BF16)
            w1r = w1p.ap().rearrange("p a b -> p (a b)")
            nc.scalar.dma_start(w1sb[:, 0:2 * 256], w1r[:, 0:2 * 256])
            xt00 = xp.tile([128, C1W], F8, name="xt", tag="xt")
            nc.sync.dma_start(xt00[:], xprep.ap()[0, 0])
            nc.scalar.dma_start(w1sb[:, 2 * 256:9 * 256],
                                w1r[:, 2 * 256:9 * 256])
            bsb = wp.tile([128, 7], F32)
            nc.scalar.dma_start(bsb[:], bprep.ap())
            nc.scalar.dma_start(w1sb[:, 9 * 256:18 * 256],
                                w1r[:, 9 * 256:18 * 256])
            w2sb = wp.tile([128, 18 * 128], BF16)
            weff = wp.tile([128, 26], F32)
            ones = wp.tile([128, BL], BF16)
            nc.gpsimd.memset(ones[:], 1.0)

            # ---------------- conv1 + incremental BN1 stats -------------
            scratch = sp.tile([128, 1600], F32)
            # Square table preload; reads bsb so the scheduler keeps the
            # 1.3us table load behind the startup DMA issues
            nc.scalar.activation(scratch[0:1, 0:1], bsb[0:1, 0:1],
                                 mybir.ActivationFunctionType.Square)
            st1i = sp.tile([128, 4], F32)    # [sum_mt0, sum_mt1, sq_mt0, sq_mt1]
            stt = sp.tile([128, 4], F32)     # per-chunk tmps
            h1sb = [hp.tile([128, 4 * PTW], BF16, name=f"h1_{mt}")
                    for mt in range(2)]
            for pt in range(NPT):
                ps = [cps.tile([128, PTW], F32, name="c1ps", tag="c1ps")
                      for _ in range(2)]
                for cb in range(4):
                    if pt == 0 and cb == 0:
                        xt = xt00
                    else:
                        xt = xp.tile([128, C1W], F8, name="xt", tag="xt")
                        c = pt * 4 + cb
                        ring = nc.sync if c % 2 == 1 else nc.scalar
                        ring.dma_start(xt[:], xprep.ap()[cb, pt])
                    if pt == 0 and cb == 1:
                        nc.scalar.dma_start(w1sb[:, 18 * 256:27 * 256],
                                            w1r[:, 18 * 256:27 * 256])
                    if pt == 0 and cb == 3:
                        nc.scalar.dma_start(w1sb[:, 27 * 256:36 * 256],
                                            w1r[:, 27 * 256:36 * 256])
                    for cnt1, (ki, kj) in enumerate(C1ORDER):
                        (il, ih), (jl, jh) = _c1rng(ki), _c1rng(kj)
                        off = C1OFF[(ki, kj)]
                        rhs = xt[:, off:off + 2 * (ih - il) * (jh - jl)]
                        kij = ki * 3 + kj
                        for mt in range(2):
                            lhsT = w1sb[:, (cb * 9 + kij) * 256 + mt * 128:
                                        (cb * 9 + kij) * 256 + (mt + 1) * 128]
                            dst = ps[mt][:].rearrange(
                                "p (n i j) -> p n i j", n=2, i=14,
                                j=14)[:, :, il:ih, jl:jh]
                            nc.tensor.matmul(dst, lhsT, rhs,
                                             start=(cb == 0 and cnt1 == 0),
                                             stop=(cb == 3 and cnt1 == 8),
                                             skip_group_check=True)
                for mt in range(2):
                    # per-chunk stats straight from psum (engines are idle
                    # during conv1; after pt3 only ~1us of tail remains)
                    nc.vector.reduce_sum(stt[:, mt:mt + 1], ps[mt][:],
                                         axis=mybir.AxisListType.X)
                    nc.scalar.activation(scratch[:, :PTW], ps[mt][:],
                                         mybir.ActivationFunctionType.Square,
                                         accum_out=stt[:, 2 + mt:3 + mt])
                    if pt == 0:
                        nc.vector.tensor_copy(st1i[:, mt::2], stt[:, mt::2])
                    else:
                        nc.vector.tensor_tensor(st1i[:, mt::2], st1i[:, mt::2],
                                                stt[:, mt::2],
                                                op=mybir.AluOpType.add)
                    # scatter psum into conv2's im2col block layout
                    pr = ps[mt][:].rearrange("p (n i j) -> p n i j",
                                             n=2, i=14, j=14)
                    for (ki, kj) in KIJ9:
                        ilo, icnt = (1, 4) if ki == 0 else (0, 5)
                        jlo, jcnt = (1, 4) if kj == 0 else (0, 5)
                        srcv = pr[:, :, 3 * ilo + ki - 1:14:3,
                                  3 * jlo + kj - 1:14:3].transpose([0, 2, 3, 1])
                        off = BLKOFF[(ki, kj)]
                        dstv = bass.AP(
                            h1sb[mt].tensor, h1sb[mt].offset + off + 2 * pt,
                            [list(h1sb[mt].ap[0]), [jcnt * 8, icnt], [8, jcnt],
                             [1, 2]])
                        nc.vector.tensor_copy(dstv, srcv)
                if pt == 1:
                    # conv2-side tensors; needed only after the BN1 mesh
                    nc.scalar.dma_start(
                        w2sb[:], w2p.ap().rearrange("p a b -> p (a b)"))
                    nc.scalar.dma_start(weff[:], weffp.ap())

            # Sqrt table prefetch while ACT is idle (pre-mesh window)
            nc.scalar.activation(scratch[0:1, 0:1], scratch[0:1, 1:2],
                                 mybir.ActivationFunctionType.Sqrt)

            # ---------------- BN1 stats exchange ------------------------
            bn1_in = dram.tile([128, 4], F32)
            nc.sync.dma_start(bn1_in[:], st1i[:])
            if USE_ALLREDUCE:
                bn1_out = dram.tile([128, 4], F32, addr_space="Shared")
                nc.gpsimd.collective_compute(
                    "AllReduce", mybir.AluOpType.add,
                    replica_groups=GROUPS,
                    ins=[bn1_in.opt()], outs=[bn1_out.opt()])
                st1 = sp.tile([128, 4], F32)
                nc.scalar.dma_start(
                    st1[:], bass.AP(bn1_out.tensor, 0, [[4, 128], [1, 4]]))
            else:
                bn1_out = dram.tile([NCORES, 128, 4], F32, addr_space="Shared")
                nc.gpsimd.collective_compute(
                    "AllGather", mybir.AluOpType.bypass,
                    replica_groups=GROUPS,
                    ins=[bn1_in.opt()], outs=[bn1_out.opt()])
                stg = sp.tile([128, NCORES * 4], F32)
                stgv = stg[:].rearrange("p (r t) -> p r t", r=NCORES)
                half_n = NCORES // 2
                nc.scalar.dma_start(
                    stgv[:, 0:half_n],
                    bass.AP(bn1_out.tensor, 0,
                            [[4, 128], [128 * 4, half_n], [1, 4]]))
                nc.sync.dma_start(
                    stgv[:, half_n:NCORES],
                    bass.AP(bn1_out.tensor, 128 * 4 * half_n,
                            [[4, 128], [128 * 4, half_n], [1, 4]]))
                stgr = stg[:].rearrange("p (r t) -> p r t", r=NCORES)
                for half in (4, 2, 1):
                    nc.vector.tensor_tensor(
                        stgr[:, 0:half], stgr[:, 0:half],
                        stgr[:, half:2 * half], op=mybir.AluOpType.add)
                st1 = stg[:, 0:4]

            # ---------------- BN coeffs helper --------------------------
            def bn_coeffs(pool, stats_sum, stats_sq, count, g_ap, b_ap, name):
                """returns (scale, shift) [p,w] tiles; stats_* are [p,w] APs"""
                p, w = stats_sum.shape
                t = pool.tile([p, 4 * w], F32, name=f"bn_{name}")
                mean, vpe, msq, sd = (t[:, i * w:(i + 1) * w] for i in range(4))
                nc.vector.tensor_scalar(mean, stats_sum, 1.0 / count, None,
                                        op0=mybir.AluOpType.mult)
                nc.vector.tensor_scalar(vpe, stats_sq, 1.0 / count, EPS,
                                        op0=mybir.AluOpType.mult,
                                        op1=mybir.AluOpType.add)
                nc.vector.tensor_tensor(msq, mean, mean, op=mybir.AluOpType.mult)
                nc.vector.tensor_tensor(vpe, vpe, msq, op=mybir.AluOpType.subtract)
                nc.scalar.activation(sd, vpe, mybir.ActivationFunctionType.Sqrt)
                nc.vector.reciprocal(msq, sd)
                co = pool.tile([p, 2 * w], F32, name=f"bnc_{name}")
                scale, shift = co[:, 0:w], co[:, w:2 * w]
                nc.vector.tensor_tensor(scale, g_ap, msq, op=mybir.AluOpType.mult)
                nc.vector.tensor_tensor(sd, mean, scale, op=mybir.AluOpType.mult)
                nc.vector.tensor_tensor(shift, b_ap, sd, op=mybir.AluOpType.subtract)
                return scale, shift

            scale1, shift1 = bn_coeffs(
                sp, st1[:, 0:2], st1[:, 2:4], B * P1,
                bsb[:, BC_BN1G:BC_BN1G + 2], bsb[:, BC_BN1B:BC_BN1B + 2], "bn1")

            # ---------------- conv2: single 18-matmul psum chain --------
            # BN1 is applied in three coarse chunks per half, ordered so the
            # first matmuls' operands go first: the PE starts ~0.3us after
            # the coeffs and the DVE stays ahead of the matmul order.
            kij_order = [(1, 1), (1, 2), (2, 1), (2, 2), (0, 1), (0, 2),
                         (1, 0), (2, 0), (0, 0)]
            c2 = c2p.tile([128, P2 * BL], F32, name="c2ps", tag="c2ps")
            c2r = c2[:].rearrange("p (i j n) -> p i j n", i=5, j=5, n=BL)
            for cb2 in range(2):
                for lo, hi in ((608, 1008), (1008, 1568), (0, 608)):
                    nc.vector.tensor_scalar(h1sb[cb2][:, lo:hi],
                                            h1sb[cb2][:, lo:hi],
                                            scale1[:, cb2:cb2 + 1],
                                            shift1[:, cb2:cb2 + 1],
                                            op0=mybir.AluOpType.mult,
                                            op1=mybir.AluOpType.add)
                for cnt, (ki, kj) in enumerate(kij_order):
                    ilo = 1 if ki == 0 else 0
                    jlo = 1 if kj == 0 else 0
                    icnt = 4 if ki == 0 else 5
                    jcnt = 4 if kj == 0 else 5
                    off = BLKOFF[(ki, kj)]
                    blk = h1sb[cb2][:, off:off + icnt * jcnt * 8]
                    dst = c2r[:, ilo:, jlo:, :]
                    lhsT = w2sb[:, (cb2 * 9 + ki * 3 + kj) * 128:
                                (cb2 * 9 + ki * 3 + kj + 1) * 128]
                    nc.tensor.matmul(dst, lhsT, blk,
                                     start=(cb2 == 0 and cnt == 0),
                                     stop=(cb2 == 1 and cnt == 8),
                                     skip_group_check=True)

            # ---------------- BN2 stats (straight from psum) ------------
            st2l = sp.tile([128, 2], F32)
            nc.vector.reduce_sum(st2l[:, 0:1], c2[:], axis=mybir.AxisListType.X)
            nc.scalar.activation(scratch[:, :BL * P2], c2[:],
                                 mybir.ActivationFunctionType.Square,
                                 accum_out=st2l[:, 1:2])
            bn2_in = dram.tile([128, 2], F32)
            nc.sync.dma_start(bn2_in[:], st2l[:])
            if USE_ALLREDUCE:
                bn2_out = dram.tile([128, 2], F32, addr_space="Shared")
                nc.gpsimd.collective_compute(
                    "AllReduce", mybir.AluOpType.add,
                    replica_groups=GROUPS,
                    ins=[bn2_in.opt()], outs=[bn2_out.opt()])
            else:
                bn2_out = dram.tile([NCORES, 128, 2], F32, addr_space="Shared")
                nc.gpsimd.collective_compute(
                    "AllGather", mybir.AluOpType.bypass,
                    replica_groups=GROUPS,
                    ins=[bn2_in.opt()], outs=[bn2_out.opt()])

            # under the mesh shadow: weff partial matvec from psum
            mvt = sp.tile([128, P2 * BL], F32)
            wb = weff[:, 0:25, None].to_broadcast([128, 25, BL])
            nc.vector.tensor_tensor(
                mvt[:].rearrange("p (i n) -> p i n", i=P2),
                c2[:].rearrange("p (i n) -> p i n", i=P2), wb,
                op=mybir.AluOpType.mult)
            Av = sp.tile([128, BL], F32)
            nc.vector.reduce_sum(Av[:], mvt[:].rearrange("p (i n) -> p n i", i=P2),
                                 axis=mybir.AxisListType.X)
            Avb = sp.tile([128, BL], BF16)
            nc.vector.tensor_copy(Avb[:], Av[:])

            # mesh result consume
            if USE_ALLREDUCE:
                st2 = sp.tile([128, 2], F32)
                nc.scalar.dma_start(
                    st2[:], bass.AP(bn2_out.tensor, 0, [[2, 128], [1, 2]]))
            else:
                stg2 = sp.tile([128, NCORES * 2], F32)
                stg2v = stg2[:].rearrange("p (r t) -> p r t", r=NCORES)
                nc.scalar.dma_start(
                    stg2v[:, 0:half_n],
                    bass.AP(bn2_out.tensor, 0,
                            [[2, 128], [128 * 2, half_n], [1, 2]]))
                nc.sync.dma_start(
                    stg2v[:, half_n:NCORES],
                    bass.AP(bn2_out.tensor, 128 * 2 * half_n,
                            [[2, 128], [128 * 2, half_n], [1, 2]]))
                stg2r = stg2[:].rearrange("p (r t) -> p r t", r=NCORES)
                for half in (4, 2, 1):
                    nc.vector.tensor_tensor(
                        stg2r[:, 0:half], stg2r[:, 0:half],
                        stg2r[:, half:2 * half], op=mybir.AluOpType.add)
                st2 = stg2[:, 0:2]

            scale2, shift2 = bn_coeffs(
                sp, st2[:, 0:1], st2[:, 1:2], B * P2,
                bsb[:, BC_BN2G:BC_BN2G + 1], bsb[:, BC_BN2B:BC_BN2B + 1], "bn2")

            # Sigmoid table prefetch: reads scale2 so the scheduler cannot
            # hoist it before coeffs2's Sqrt (which would evict the Sqrt
            # table); its load overlaps the closing DVE/PE chain
            nc.scalar.activation(scratch[0:1, 0:1], scale2[0:1, 0:1],
                                 mybir.ActivationFunctionType.Sigmoid)

            # ---------------- collapsed MLP finish ----------------------
            # z[n] = sum_c s2[c]*A[c,n] + sum_c t2[c]*rowsum_weff[c]
            s2b = sp.tile([128, 1], BF16)
            nc.vector.tensor_copy(s2b[:], scale2)
            vsh = wp.tile([128, 1], BF16)
            nc.vector.tensor_tensor(vsh[:], shift2, weff[:, 25:26],
                                    op=mybir.AluOpType.mult)
            zps = zp.tile([1, BL], F32)
            nc.tensor.matmul(zps[:], s2b[:], Avb[:], start=True, stop=False)
            nc.tensor.matmul(zps[:], vsh[:], ones[:], start=False, stop=True)
            osb = sp.tile([1, BL], F32)
            nc.scalar.activation(osb[:], zps[:],
                                 mybir.ActivationFunctionType.Sigmoid,
                                 bias=bsb[0:1, BC_BEFF:BC_BEFF + 1])
            nc.sync.dma_start(bass.AP(out, 0, [[1, 1], [1, BL]]), osb[:])

    nc.compile()
    return nc


# ----------------------------------------------------------------------------
# host-side input prep
# ----------------------------------------------------------------------------

def _prep_inputs(inputs):
    import ml_dtypes
    f = np.float32
    bf = ml_dtypes.bfloat16
    f8 = ml_dtypes.float8_e4m3
    x = np.asarray(inputs["x"], dtype=f)

    # conv1 patches in fp8 (halves the dominant DMA stream; the PE upcasts
    # the fp8 rhs against bf16 weights), boundary-trimmed: per kij only the
    # valid output positions ship, so no zero padding moves or matmuls.
    # layout per core: [4cb, 4pt, 128c, C1W] with [n2, i, j] blocks in C1ORDER.
    x8 = np.asarray(x, dtype=f8).reshape(B, 4, 128, 40, 40)
    xs = np.empty((4, B, 128, 1600), dtype=f8)      # per-sample trimmed cols
    for ki, kj in C1ORDER:
        (il, ih), (jl, jh) = _c1rng(ki), _c1rng(kj)
        ri = [3 * i + ki - 1 for i in range(il, ih)]
        rj = [3 * j + kj - 1 for j in range(jl, jh)]
        w = (ih - il) * (jh - jl)
        o = C1OFF[(ki, kj)] // 2
        blk = x8[:, :, :, ri, :][:, :, :, :, rj]    # [B, cb, c, i, j]
        xs[:, :, :, o:o + w] = blk.transpose(1, 0, 2, 3, 4).reshape(4, B, 128, w)
    xs6 = xs.reshape(4, NCORES, NPT, 2, 128, 1600)  # [cb, r, pt, n2, c, cols]
    xnew = np.empty((NCORES, 4, NPT, 128, C1W), dtype=f8)
    for ki, kj in C1ORDER:
        (il, ih), (jl, jh) = _c1rng(ki), _c1rng(kj)
        w = (ih - il) * (jh - jl)
        o = C1OFF[(ki, kj)]
        piece = xs6[:, :, :, :, :, o // 2:o // 2 + w]
        xnew[:, :, :, :, o:o + 2 * w] = piece.transpose(
            1, 0, 2, 4, 3, 5).reshape(NCORES, 4, NPT, 128, 2 * w)

    w1 = np.asarray(inputs["conv1_w"], dtype=f)          # [256, 512, 3, 3]
    w1p = np.ascontiguousarray(
        w1.reshape(256, 4, 128, 9).transpose(2, 1, 3, 0)).reshape(128, 36, 256).astype(bf)
    w2 = np.asarray(inputs["conv2_w"], dtype=f)          # [128, 256, 3, 3]
    w2p = np.ascontiguousarray(
        w2.reshape(128, 2, 128, 9).transpose(2, 1, 3, 0)).reshape(128, 18, 128).astype(bf)

    # compose the 12 affine layers (no nonlinearities) into [3200] + scalar
    M = np.asarray(inputs["w14"], dtype=np.float64)      # [1, 2]
    beff = np.asarray(inputs["b14"], dtype=np.float64).copy()  # [1]
    for li in range(13, 2, -1):                          # w13 .. w3
        beff += M @ np.asarray(inputs[f"b{li}"], dtype=np.float64)
        M = M @ np.asarray(inputs[f"w{li}"], dtype=np.float64)
    weff = M.reshape(3200).astype(f)                     # order f = c*25 + ij
    w2d = weff.reshape(128, 25)
    weffp = np.zeros((128, 26), dtype=f)
    weffp[:, 0:25] = w2d
    weffp[:, 25] = w2d.sum(axis=1)
    beff_f = float(beff[0])

    bn1_g = np.asarray(inputs["bn1_g"], dtype=f)
    bn1_b = np.asarray(inputs["bn1_b"], dtype=f)
    bn2_g = np.asarray(inputs["bn2_g"], dtype=f)
    bn2_b = np.asarray(inputs["bn2_b"], dtype=f)

    bp = np.zeros((128, 7), dtype=f)
    bp[:, 0:2] = bn1_g.reshape(2, 128).T
    bp[:, 2:4] = bn1_b.reshape(2, 128).T
    bp[:, 4] = bn2_g
    bp[:, 5] = bn2_b
    bp[0, 6] = beff_f

    in_maps = []
    for r in range(NCORES):
        in_maps.append({
            "xprep": np.ascontiguousarray(xnew[r]), "w1p": w1p, "w2p": w2p,
            "weffp": weffp, "bprep": bp,
        })
    return in_maps


def _install_ntff_shim():
    """Register the NTFF profile hook concourse looks for under axon.

    The container's `antenv` package lacks `axon_hooks`; recreate it with
    direct ctypes calls into libaxon_pjrt.so (same ABI the axon boot
    script uses). Returns True if the hook is usable."""
    import contextlib
    import ctypes
    import types

    try:
        from antenv.axon_hooks import get_axon_ntff_profile_hook  # noqa: F401
        return True
    except ImportError:
        pass
    so_path = "/opt/axon/libaxon_pjrt.so"
    if not os.path.exists(so_path):
        return False
    lib = ctypes.CDLL(so_path)
    if not hasattr(lib, "axon_start_nrt_profile"):
        return False
    lib.axon_start_nrt_profile.argtypes = [ctypes.POINTER(ctypes.c_int64),
                                           ctypes.c_size_t]
    lib.axon_start_nrt_profile.restype = ctypes.c_int64
    lib.axon_stop_nrt_profile.argtypes = [ctypes.c_char_p]
    lib.axon_stop_nrt_profile.restype = ctypes.c_int64

    @contextlib.contextmanager
    def _hook(output_dir, device_ids):
        import jax
        jax.devices()
        if device_ids:
            ids = (ctypes.c_int64 * len(device_ids))(*device_ids)
            rc = lib.axon_start_nrt_profile(ids, len(device_ids))
        else:
            rc = lib.axon_start_nrt_profile(None, 0)
        if rc != 0:
            raise RuntimeError(f"axon_start_nrt_profile rc={rc}")
        try:
            yield
        finally:
            n = lib.axon_stop_nrt_profile(str(output_dir).encode())
            if n < 0:
                raise RuntimeError(f"axon_stop_nrt_profile rc={n}")
            print(f"profile: {n} file(s) written to {output_dir}",
                  file=sys.stderr)

    mod = types.ModuleType("antenv.axon_hooks")
    mod.get_axon_ntff_profile_hook = lambda: _hook
    mod.set_axon_ntff_profile_hook = lambda h: None
    import antenv
    antenv.axon_hooks = mod
    sys.modules["antenv.axon_hooks"] = mod
    return True


def kernel(**inputs):
    if "nc" not in _CACHE:
        _CACHE["nc"] = _build()
    nc = _CACHE["nc"]
    in_maps = _prep_inputs(inputs)
    trace = bool(int(os.environ.get("KERNEL_TRACE", "0")))
    if trace:
        trace = _install_ntff_shim()
    res = run_bass_kernel_spmd(nc, in_maps, core_ids=list(range(NCORES)),
                               trace=trace)
    _CACHE["last_result"] = res
    return np.concatenate([res.results[r]["out"] for r in range(NCORES)], axis=0)


# revision 35
# speedup vs baseline: 1.0144x; 1.0144x over previous
"""Trainium2 Bass kernel for nn_DomainDiscriminator.

Network: conv(512->256,k3,s3,p1) -> BN -> conv(256->128,k3,s3,p1) -> BN
         -> reshape -> 12-layer MLP (3200->...->1, no nonlinearities) -> sigmoid.
Input x: [64, 512, 40, 40] f32.  Output: [64, 1] f32.

Strategy (8 NeuronCores):
 - Data-parallel batch shard (8 per core) for the convs.
 - stride==kernel==3 convs are non-overlapping patch matmuls. Conv1 patches are
   built host-side (space-to-depth, free); conv2 patches are read straight out
   of SBUF with strided access patterns (boundary-split matmuls, no im2col).
 - Training-mode BN: conv bias is absorbed exactly by BN; per-channel batch
   stats are exchanged with tiny (1-2KB) AllReduces.
 - The 12 linear layers have no activations between them, so they compose on
   the host (fp64) into a single [3200] vector + scalar bias; the device
   computes a per-channel partial matvec under the BN2 mesh shadow and
   finishes with two tiny matmuls + sigmoid on its own batch shard; the host
   concatenates the 8 per-core output shards.
 - Convs run in bf16 (BN re-normalizes, keeping error ~3e-3).

Latency structure (per core):
 - warmup collective triggered at t~0 straight from an ExternalInput (no
   staging DMA) so the ~60us TOPSP cold start completes before BN1's mesh.
 - startup DMAs fan out over 4 HWDGE rings (tensor/vector/sync/gpsimd) with
   the first x chunk split so the first matmul issues ~5us earlier.
 - BN1 stats are accumulated incrementally per conv1 psum tile on the
   otherwise-idle Vector/ACT engines, so the stats post ~3us after conv1's
   last matmul.
 - conv2 runs as a single 18-matmul psum chain; its stats are read straight
   from PSUM and the weff partial matvec happens under the BN2 mesh.
 - ACT table loads (Sqrt/Sigmoid) are prefetched into idle windows via dummy
   activations so no table load sits on the critical path.
"""

import os
import sys

sys.path.insert(0, "/opt/trn_rl_repo")

import numpy as np

import concourse.bass as bass
import concourse.mybir as mybir
import concourse.tile as tile
from concourse import bacc
from concourse.bass_utils import run_bass_kernel_spmd

F32 = mybir.dt.float32
BF16 = mybir.dt.bfloat16
F8 = mybir.dt.float8e4

NCORES = 8
BL = 8              # batch per core
B = 64              # full batch
EPS = 1e-5

# conv1: [BL,512,40,40] -> [BL,256,14,14]; conv2: -> [BL,128,5,5]
P1 = 196            # 14*14 positions
P2 = 25             # 5*5 positions
NPT = 4             # conv1 psum tiles (2 batch each)
PTW = 2 * P1        # 392 columns per conv1 psum tile

USE_ALLREDUCE = bool(int(os.environ.get("KERNEL_ALLREDUCE", "0")))

_CACHE = {}

KIJ9 = [(ki, kj) for ki in range(3) for kj in range(3)]
# conv2 im2col block offsets within an h1 patch tile [128, 1568]
BLKOFF = {}
_o = 0
for _ki, _kj in KIJ9:
    BLKOFF[(_ki, _kj)] = _o
    _o += (4 if _ki == 0 else 5) * (4 if _kj == 0 else 5) * 8
assert _o == 1568

# conv1 boundary-trimmed im2col: per kij only the valid output rows/cols are
# shipped and matmul'd ((1,1) goes first so psum start= zeroes the full tile)
C1ORDER = [(1, 1), (0, 1), (1, 0), (1, 2), (2, 1), (0, 0), (0, 2), (2, 0), (2, 2)]


def _c1rng(k):
    # valid output index range [lo, hi) for kernel offset k (stride 3, pad 1)
    return (1, 14) if k == 0 else ((0, 13) if k == 2 else (0, 14))


C1OFF = {}
C1W = 0
for _ki, _kj in C1ORDER:
    C1OFF[(_ki, _kj)] = C1W
    (_il, _ih), (_jl, _jh) = _c1rng(_ki), _c1rng(_kj)
    C1W += 2 * (_ih - _il) * (_jh - _jl)
assert C1W == 3200


# ----------------------------------------------------------------------------
# device program
# ----------------------------------------------------------------------------

def _build():
    nc = bacc.Bacc("TRN2", target_bir_lowering=False, debug=False,
                   enable_asserts=False, num_devices=NCORES)

    xprep = nc.dram_tensor("xprep", [4, NPT, 128, C1W], F8,
                           kind="ExternalInput")
    w1p = nc.dram_tensor("w1p", [128, 36, 256], BF16, kind="ExternalInput")
    w2p = nc.dram_tensor("w2p", [128, 18, 128], BF16, kind="ExternalInput")
    weffp = nc.dram_tensor("weffp", [128, 26], F32, kind="ExternalInput")
    bprep = nc.dram_tensor("bprep", [128, 7], F32, kind="ExternalInput")
    out = nc.dram_tensor("out", [BL, 1], F32, kind="ExternalOutput")

    # bprep columns: bn1_g (2), bn1_b (2), bn2_g, bn2_b, beff(row 0)
    BC_BN1G, BC_BN1B, BC_BN2G, BC_BN2B, BC_BEFF = 0, 2, 4, 5, 6
    GROUPS = [list(range(NCORES))]

    with tile.TileContext(nc) as tc:
        with tc.tile_pool(name="wp", bufs=1) as wp, \
             tc.tile_pool(name="xp", bufs=6) as xp, \
             tc.tile_pool(name="hp", bufs=1) as hp, \
             tc.tile_pool(name="sp", bufs=1) as sp, \
             tc.tile_pool(name="cps", bufs=4, space="PSUM") as cps, \
             tc.tile_pool(name="c2p", bufs=1, space="PSUM") as c2p, \
             tc.tile_pool(name="zp", bufs=1, space="PSUM") as zp, \
             tc.tile_pool(name="dram", bufs=1, space="DRAM") as dram:

            # ---------------- collective warm-up ------------------------
            # TOPSP's first collective has a ~60us cold start; trigger it
            # as early as possible (collectives can't read IO tensors, so
            # bounce a tiny input through DRAM scratch first) so the mesh
            # is warm before the BN1 stats exchange (~66us).
            warm_in = dram.tile([1, 4], F32)
            warm_out = dram.tile([NCORES, 1, 4], F32, addr_space="Shared")
            nc.sync.dma_start(warm_in[:], bprep.ap()[0:1, 0:4])
            nc.gpsimd.collective_compute(
                "AllGather", mybir.AluOpType.bypass,
                replica_groups=GROUPS,
                ins=[warm_in.opt()], outs=[warm_out.opt()])

            # ---------------- priority loads ----------------------------
            # Only SP(sync) and Activation(scalar) are fast HWDGE rings;
            # the gpsimd ring is a high-latency SW DGE and only carries
            # late bulk (w2/weff and the last x chunks). DMA completion
            # semaphores fire ~1.5-3us after the data and serialize per
            # ring, so the first-matmul dependencies (w1 cb0 + xt00) are
            # single whole DMAs, one per fast ring.
            w1sb = wp.tile([128, 36 * 256], BF16)
            w1r = w1p.ap().rearrange("p a b -> p (a b)")
            nc.scalar.dma_start(w1sb[:, 0:2 * 256], w1r[:, 0:2 * 256])
            xt00 = xp.tile([128, C1W], F8, name="xt", tag="xt")
            nc.sync.dma_start(xt00[:], xprep.ap()[0, 0])
            nc.scalar.dma_start(w1sb[:, 2 * 256:9 * 256],
                                w1r[:, 2 * 256:9 * 256])
            bsb = wp.tile([128, 7], F32)
            nc.scalar.dma_start(bsb[:], bprep.ap())
            nc.scalar.dma_start(w1sb[:, 9 * 256:18 * 256],
                                w1r[:, 9 * 256:18 * 256])
            w2sb = wp.tile([128, 18 * 128], BF16)
            weff = wp.tile([128, 26], F32)
            ones = wp.tile([128, BL], BF16)
            nc.gpsimd.memset(ones[:], 1.0)

            # ---------------- conv1 + incremental BN1 stats -------------
            scratch = sp.tile([128, 1600], F32)
            # Square table preload; reads bsb so the scheduler keeps the
            # 1.3us table load behind the startup DMA issues
            nc.scalar.activation(scratch[0:1, 0:1], bsb[0:1, 0:1],
                                 mybir.ActivationFunctionType.Square)
            st1i = sp.tile([128, 4], F32)    # [sum_mt0, sum_mt1, sq_mt0, sq_mt1]
            stt = sp.tile([128, 4], F32)     # per-chunk tmps
            h1sb = [hp.tile([128, 4 * PTW], BF16, name=f"h1_{mt}")
                    for mt in range(2)]
            for pt in range(NPT):
                ps = [cps.tile([128, PTW], F32, name="c1ps", tag="c1ps")
                      for _ in range(2)]
                for cb in range(4):
                    if pt == 0 and cb == 0:
                        xt = xt00
                    else:
                        xt = xp.tile([128, C1W], F8, name="xt", tag="xt")
                        c = pt * 4 + cb
                        ring = nc.sync if c % 2 == 1 else nc.scalar
                        ring.dma_start(xt[:], xprep.ap()[cb, pt])
                    if pt == 0 and cb == 1:
                        nc.scalar.dma_start(w1sb[:, 18 * 256:27 * 256],
                                            w1r[:, 18 * 256:27 * 256])
                    if pt == 0 and cb == 3:
                        nc.scalar.dma_start(w1sb[:, 27 * 256:36 * 256],
                                            w1r[:, 27 * 256:36 * 256])
                    for cnt1, (ki, kj) in enumerate(C1ORDER):
                        (il, ih), (jl, jh) = _c1rng(ki), _c1rng(kj)
                        off = C1OFF[(ki, kj)]
                        rhs = xt[:, off:off + 2 * (ih - il) * (jh - jl)]
                        kij = ki * 3 + kj
                        for mt in range(2):
                            lhsT = w1sb[:, (cb * 9 + kij) * 256 + mt * 128:
                                        (cb * 9 + kij) * 256 + (mt + 1) * 128]
                            dst = ps[mt][:].rearrange(
                                "p (n i j) -> p n i j", n=2, i=14,
                                j=14)[:, :, il:ih, jl:jh]
                            nc.tensor.matmul(dst, lhsT, rhs,
                                             start=(cb == 0 and cnt1 == 0),
                                             stop=(cb == 3 and cnt1 == 8),
                                             skip_group_check=True)
                for mt in range(2):
                    # per-chunk stats straight from psum (engines are idle
                    # during conv1; after pt3 only ~1us of tail remains)
                    nc.vector.reduce_sum(stt[:, mt:mt + 1], ps[mt][:],
                                         axis=mybir.AxisListType.X)
                    nc.scalar.activation(scratch[:, :PTW], ps[mt][:],
                                         mybir.ActivationFunctionType.Square,
                                         accum_out=stt[:, 2 + mt:3 + mt])
                    if pt == 0:
                        nc.vector.tensor_copy(st1i[:, mt::2], stt[:, mt::2])
                    else:
                        nc.vector.tensor_tensor(st1i[:, mt::2], st1i[:, mt::2],
                                                stt[:, mt::2],
                                                op=mybir.AluOpType.add)
                    # scatter psum into conv2's im2col block layout
                    pr = ps[mt][:].rearrange("p (n i j) -> p n i j",
                                             n=2, i=14, j=14)
                    for (ki, kj) in KIJ9:
                        ilo, icnt = (1, 4) if ki == 0 else (0, 5)
                        jlo, jcnt = (1, 4) if kj == 0 else (0, 5)
                        srcv = pr[:, :, 3 * ilo + ki - 1:14:3,
                                  3 * jlo + kj - 1:14:3].transpose([0, 2, 3, 1])
                        off = BLKOFF[(ki, kj)]
                        dstv = bass.AP(
                            h1sb[mt].tensor, h1sb[mt].offset + off + 2 * pt,
                            [list(h1sb[mt].ap[0]), [jcnt * 8, icnt], [8, jcnt],
                             [1, 2]])
                        nc.vector.tensor_copy(dstv, srcv)
                if pt == 1:
                    # conv2-side tensors; needed only after the BN1 mesh
                    nc.scalar.dma_start(
                        w2sb[:], w2p.ap().rearrange("p a b -> p (a b)"))
                    nc.scalar.dma_start(weff[:], weffp.ap())

            # Sqrt table prefetch while ACT is idle (pre-mesh window)
            nc.scalar.activation(scratch[0:1, 0:1], scratch[0:1, 1:2],
                                 mybir.ActivationFunctionType.Sqrt)

            # ---------------- BN1 stats exchange ------------------------
            bn1_in = dram.tile([128, 4], F32)
            nc.sync.dma_start(bn1_in[:], st1i[:])
            if USE_ALLREDUCE:
                bn1_out = dram.tile([128, 4], F32, addr_space="Shared")
                nc.gpsimd.collective_compute(
                    "AllReduce", mybir.AluOpType.add,
                    replica_groups=GROUPS,
                    ins=[bn1_in.opt()], outs=[bn1_out.opt()])
                st1 = sp.tile([128, 4], F32)
                nc.scalar.dma_start(
                    st1[:], bass.AP(bn1_out.tensor, 0, [[4, 128], [1, 4]]))
            else:
                bn1_out = dram.tile([NCORES, 128, 4], F32, addr_space="Shared")
                nc.gpsimd.collective_compute(
                    "AllGather", mybir.AluOpType.bypass,
                    replica_groups=GROUPS,
                    ins=[bn1_in.opt()], outs=[bn1_out.opt()])
                stg = sp.tile([128, NCORES * 4], F32)
                stgv = stg[:].rearrange("p (r t) -> p r t", r=NCORES)
                half_n = NCORES // 2
                nc.scalar.dma_start(
                    stgv[:, 0:half_n],
                    bass.AP(bn1_out.tensor, 0,
                            [[4, 128], [128 * 4, half_n], [1, 4]]))
                nc.sync.dma_start(
                    stgv[:, half_n:NCORES],
                    bass.AP(bn1_out.tensor, 128 * 4 * half_n,
                            [[4, 128], [128 * 4, half_n], [1, 4]]))
                stgr = stg[:].rearrange("p (r t) -> p r t", r=NCORES)
                for half in (4, 2, 1):
                    nc.vector.tensor_tensor(
                        stgr[:, 0:half], stgr[:, 0:half],
                        stgr[:, half:2 * half], op=mybir.AluOpType.add)
                st1 = stg[:, 0:4]

            # ---------------- BN coeffs helper --------------------------
            def bn_coeffs(pool, stats_sum, stats_sq, count, g_ap, b_ap, name):
                """returns (scale, shift) [p,w] tiles; stats_* are [p,w] APs"""
                p, w = stats_sum.shape
                t = pool.tile([p, 4 * w], F32, name=f"bn_{name}")
                mean, vpe, msq, sd = (t[:, i * w:(i + 1) * w] for i in range(4))
                nc.vector.tensor_scalar(mean, stats_sum, 1.0 / count, None,
                                        op0=mybir.AluOpType.mult)
                nc.vector.tensor_scalar(vpe, stats_sq, 1.0 / count, EPS,
                                        op0=mybir.AluOpType.mult,
                                        op1=mybir.AluOpType.add)
                nc.vector.tensor_tensor(msq, mean, mean, op=mybir.AluOpType.mult)
                nc.vector.tensor_tensor(vpe, vpe, msq, op=mybir.AluOpType.subtract)
                nc.scalar.activation(sd, vpe, mybir.ActivationFunctionType.Sqrt)
                nc.vector.reciprocal(msq, sd)
                co = pool.tile([p, 2 * w], F32, name=f"bnc_{name}")
                scale, shift = co[:, 0:w], co[:, w:2 * w]
                nc.vector.tensor_tensor(scale, g_ap, msq, op=mybir.AluOpType.mult)
                nc.vector.tensor_tensor(sd, mean, scale, op=mybir.AluOpType.mult)
                nc.vector.tensor_tensor(shift, b_ap, sd, op=mybir.AluOpType.subtract)
                return scale, shift

            scale1, shift1 = bn_coeffs(
                sp, st1[:, 0:2], st1[:, 2:4], B * P1,
                bsb[:, BC_BN1G:BC_BN1G + 2], bsb[:, BC_BN1B:BC_BN1B + 2], "bn1")

            # ---------------- conv2: single 18-matmul psum chain --------
            # BN1 is applied in three coarse chunks per half, ordered so the
            # first matmuls' operands go first: the PE starts ~0.3us after
            # the coeffs and the DVE stays ahead of the matmul order.
            kij_order = [(1, 1), (1, 2), (2, 1), (2, 2), (0, 1), (0, 2),
                         (1, 0), (2, 0), (0, 0)]
            c2 = c2p.tile([128, P2 * BL], F32, name="c2ps", tag="c2ps")
            c2r = c2[:].rearrange("p (i j n) -> p i j n", i=5, j=5, n=BL)
            for cb2 in range(2):
                for lo, hi in ((608, 1008), (1008, 1568), (0, 608)):
                    nc.vector.tensor_scalar(h1sb[cb2][:, lo:hi],
                                            h1sb[cb2][:, lo:hi],
                                            scale1[:, cb2:cb2 + 1],
                                            shift1[:, cb2:cb2 + 1],
                                            op0=mybir.AluOpType.mult,
                                            op1=mybir.AluOpType.add)
                for cnt, (ki, kj) in enumerate(kij_order):
                    ilo = 1 if ki == 0 else 0
                    jlo = 1 if kj == 0 else 0
                    icnt = 4 if ki == 0 else 5
                    jcnt = 4 if kj == 0 else 5
                    off = BLKOFF[(ki, kj)]
                    blk = h1sb[cb2][:, off:off + icnt * jcnt * 8]
                    dst = c2r[:, ilo:, jlo:, :]
                    lhsT = w2sb[:, (cb2 * 9 + ki * 3 + kj) * 128:
                                (cb2 * 9 + ki * 3 + kj + 1) * 128]
                    nc.tensor.matmul(dst, lhsT, blk,
                                     start=(cb2 == 0 and cnt == 0),
                                     stop=(cb2 == 1 and cnt == 8),
                                     skip_group_check=True)

            # ---------------- BN2 stats (straight from psum) ------------
            st2l = sp.tile([128, 2], F32)
            nc.vector.reduce_sum(st2l[:, 0:1], c2[:], axis=mybir.AxisListType.X)
            nc.scalar.activation(scratch[:, :BL * P2], c2[:],
                                 mybir.ActivationFunctionType.Square,
                                 accum_out=st2l[:, 1:2])
            bn2_in = dram.tile([128, 2], F32)
            nc.sync.dma_start(bn2_in[:], st2l[:])
            if USE_ALLREDUCE:
                bn2_out = dram.tile([128, 2], F32, addr_space="Shared")
                nc.gpsimd.collective_compute(
                    "AllReduce", mybir.AluOpType.add,
                    replica_groups=GROUPS,
                    ins=[bn2_in.opt()], outs=[bn2_out.opt()])
            else:
                bn2_out = dram.tile([NCORES, 128, 2], F32, addr_space="Shared")
                nc.gpsimd.collective_compute(
                    "AllGather", mybir.AluOpType.bypass,
                    replica_groups=GROUPS,
                    ins=[bn2_in.opt()], outs=[bn2_out.opt()])

            # under the mesh shadow: weff partial matvec from psum
            mvt = sp.tile([128, P2 * BL], F32)
            wb = weff[:, 0:25, None].to_broadcast([128, 25, BL])
            nc.vector.tensor_tensor(
                mvt[:].rearrange("p (i n) -> p i n", i=P2),
                c2[:].rearrange("p (i n) -> p i n", i=P2), wb,
                op=mybir.AluOpType.mult)
            Av = sp.tile([128, BL], F32)
            nc.vector.reduce_sum(Av[:], mvt[:].rearrange("p (i n) -> p n i", i=P2),
                                 axis=mybir.AxisListType.X)
            Avb = sp.tile([128, BL], BF16)
            nc.vector.tensor_copy(Avb[:], Av[:])

            # mesh result consume
            if USE_ALLREDUCE:
                st2 = sp.tile([128, 2], F32)
                nc.scalar.dma_start(
                    st2[:], bass.AP(bn2_out.tensor, 0, [[2, 128], [1, 2]]))
            else:
                stg2 = sp.tile([128, NCORES * 2], F32)
                stg2v = stg2[:].rearrange("p (r t) -> p r t", r=NCORES)
                nc.scalar.dma_start(
                    stg2v[:, 0:half_n],
                    bass.AP(bn2_out.tensor, 0,
                            [[2, 128], [128 * 2, half_n], [1, 2]]))
                nc.sync.dma_start(
                    stg2v[:, half_n:NCORES],
                    bass.AP(bn2_out.tensor, 128 * 2 * half_n,
                            [[2, 128], [128 * 2, half_n], [1, 2]]))
                stg2r = stg2[:].rearrange("p (r t) -> p r t", r=NCORES)
                for half in (4, 2, 1):
                    nc.vector.tensor_tensor(
                        stg2r[:, 0:half], stg2r[:, 0:half],
                        stg2r[:, half:2 * half], op=mybir.AluOpType.add)
                st2 = stg2[:, 0:2]

            scale2, shift2 = bn_coeffs(
                sp, st2[:, 0:1], st2[:, 1:2], B * P2,
                bsb[:, BC_BN2G:BC_BN2G + 1], bsb[:, BC_BN2B:BC_BN2B + 1], "bn2")

            # Sigmoid table prefetch: reads scale2 so the scheduler cannot
            # hoist it before coeffs2's Sqrt (which would evict the Sqrt
            # table); its load overlaps the closing DVE/PE chain
            nc.scalar.activation(scratch[0:1, 0:1], scale2[0:1, 0:1],
                                 mybir.ActivationFunctionType.Sigmoid)

            # ---------------- collapsed MLP finish ----------------------
            # z[n] = sum_c s2[c]*A[c,n] + sum_c t2[c]*rowsum_weff[c]
            s2b = sp.tile([128, 1], BF16)
            nc.vector.tensor_copy(s2b[:], scale2)
            vsh = wp.tile([128, 1], BF16)
            nc.vector.tensor_tensor(vsh[:], shift2, weff[:, 25:26],
                                    op=mybir.AluOpType.mult)
            zps = zp.tile([1, BL], F32)
            nc.tensor.matmul(zps[:], s2b[:], Avb[:], start=True, stop=False)
            nc.tensor.matmul(zps[:], vsh[:], ones[:], start=False, stop=True)
            osb = sp.tile([1, BL], F32)
            nc.scalar.activation(osb[:], zps[:],
                                 mybir.ActivationFunctionType.Sigmoid,
                                 bias=bsb[0:1, BC_BEFF:BC_BEFF + 1])
            nc.sync.dma_start(bass.AP(out, 0, [[1, 1], [1, BL]]), osb[:])

    nc.compile()
    return nc


# ----------------------------------------------------------------------------
# host-side input prep
# ----------------------------------------------------------------------------

def _prep_inputs(inputs):
    import ml_dtypes
    f = np.float32
    bf = ml_dtypes.bfloat16
    f8 = ml_dtypes.float8_e4m3
    x = np.asarray(inputs["x"], dtype=f)

    # conv1 patches in fp8 (halves the dominant DMA stream; the PE upcasts
    # the fp8 rhs against bf16 weights), boundary-trimmed: per kij only the
    # valid output positions ship, so no zero padding moves or matmuls.
    # layout per core: [4cb, 4pt, 128c, C1W] with [n2, i, j] blocks in C1ORDER.
    x8 = np.asarray(x, dtype=f8).reshape(B, 4, 128, 40, 40)
    xs = np.empty((4, B, 128, 1600), dtype=f8)      # per-sample trimmed cols
    for ki, kj in C1ORDER:
        (il, ih), (jl, jh) = _c1rng(ki), _c1rng(kj)
        ri = [3 * i + ki - 1 for i in range(il, ih)]
        rj = [3 * j + kj - 1 for j in range(jl, jh)]
        w = (ih - il) * (jh - jl)
        o = C1OFF[(ki, kj)] // 2
        blk = x8[:, :, :, ri, :][:, :, :, :, rj]    # [B, cb, c, i, j]
        xs[:, :, :, o:o + w] = blk.transpose(1, 0, 2, 3, 4).reshape(4, B, 128, w)
    xs6 = xs.reshape(4, NCORES, NPT, 2, 128, 1600)  # [cb, r, pt, n2, c, cols]
    xnew = np.empty((NCORES, 4, NPT, 128, C1W), dtype=f8)
    for ki, kj in C1ORDER:
        (il, ih), (jl, jh) = _c1rng(ki), _c1rng(kj)
        w = (ih - il) * (jh - jl)
        o = C1OFF[(ki, kj)]
        piece = xs6[:, :, :, :, :, o // 2:o // 2 + w]
        xnew[:, :, :, :, o:o + 2 * w] = piece.transpose(
            1, 0, 2, 4, 3, 5).reshape(NCORES, 4, NPT, 128, 2 * w)

    w1 = np.asarray(inputs["conv1_w"], dtype=f)          # [256, 512, 3, 3]
    w1p = np.ascontiguousarray(
        w1.reshape(256, 4, 128, 9).transpose(2, 1, 3, 0)).reshape(128, 36, 256).astype(bf)
    w2 = np.asarray(inputs["conv2_w"], dtype=f)          # [128, 256, 3, 3]
    w2p = np.ascontiguousarray(
        w2.reshape(128, 2, 128, 9).transpose(2, 1, 3, 0)).reshape(128, 18, 128).astype(bf)

    # compose the 12 affine layers (no nonlinearities) into [3200] + scalar
    M = np.asarray(inputs["w14"], dtype=np.float64)      # [1, 2]
    beff = np.asarray(inputs["b14"], dtype=np.float64).copy()  # [1]
    for li in range(13, 2, -1):                          # w13 .. w3
        beff += M @ np.asarray(inputs[f"b{li}"], dtype=np.float64)
        M = M @ np.asarray(inputs[f"w{li}"], dtype=np.float64)
    weff = M.reshape(3200).astype(f)                     # order f = c*25 + ij
    w2d = weff.reshape(128, 25)
    weffp = np.zeros((128, 26), dtype=f)
    weffp[:, 0:25] = w2d
    weffp[:, 25] = w2d.sum(axis=1)
    beff_f = float(beff[0])

    bn1_g = np.asarray(inputs["bn1_g"], dtype=f)
    bn1_b = np.asarray(inputs["bn1_b"], dtype=f)
    bn2_g = np.asarray(inputs["bn2_g"], dtype=f)
    bn2_b = np.asarray(inputs["bn2_b"], dtype=f)

    bp = np.zeros((128, 7), dtype=f)
    bp[:, 0:2] = bn1_g.reshape(2, 128).T
    bp[:, 2:4] = bn1_b.reshape(2, 128).T
    bp[:, 4] = bn2_g
    bp[:, 5] = bn2_b
    bp[0, 6] = beff_f

    in_maps = []
    for r in range(NCORES):
        in_maps.append({
            "xprep": np.ascontiguousarray(xnew[r]), "w1p": w1p, "w2p": w2p,
            "weffp": weffp, "bprep": bp,
        })
    return in_maps


def _install_ntff_shim():
    """Register the NTFF profile hook concourse looks for under axon.

    The container's `antenv` package lacks `axon_hooks`; recreate it with
    direct ctypes calls into libaxon_pjrt.so (same ABI the axon boot
    script uses). Returns True if the hook is usable."""
    import contextlib
    import ctypes
    import types

    try:
        from antenv.axon_hooks import get_axon_ntff_profile_hook  # noqa: F401
        return True
    except ImportError:
        pass
    so_path = "/opt/axon/libaxon_pjrt.so"
    if not os.path.exists(so_path):
        return False
    lib = ctypes.CDLL(so_path)
    if not hasattr(lib, "axon_start_nrt_profile"):
        return False
    lib.axon_start_nrt_profile.argtypes = [ctypes.POINTER(ctypes.c_int64),
                                           ctypes.c_size_t]
    lib.axon_start_nrt_profile.restype = ctypes.c_int64
    lib.axon_stop_nrt_profile.argtypes = [ctypes.c_char_p]
    lib.axon_stop_nrt_profile.restype = ctypes.c_int64

    @contextlib.contextmanager
    def _hook(output_dir, device_ids):
        import jax
        jax.devices()
        if device_ids:
            ids = (ctypes.c_int64 * len(device_ids))(*device_ids)
            rc = lib.axon_start_nrt_profile(ids, len(device_ids))
        else:
            rc = lib.axon_start_nrt_profile(None, 0)
        if rc != 0:
            raise RuntimeError(f"axon_start_nrt_profile rc={rc}")
        try:
            yield
        finally:
            n = lib.axon_stop_nrt_profile(str(output_dir).encode())
            if n < 0:
                raise RuntimeError(f"axon_stop_nrt_profile rc={n}")
            print(f"profile: {n} file(s) written to {output_dir}",
                  file=sys.stderr)

    mod = types.ModuleType("antenv.axon_hooks")
    mod.get_axon_ntff_profile_hook = lambda: _hook
    mod.set_axon_ntff_profile_hook = lambda h: None
    import antenv
    antenv.axon_hooks = mod
    sys.modules["antenv.axon_hooks"] = mod
    return True


def kernel(**inputs):
    if "nc" not in _CACHE:
        _CACHE["nc"] = _build()
    nc = _CACHE["nc"]
    in_maps = _prep_inputs(inputs)
    trace = bool(int(os.environ.get("KERNEL_TRACE", "0")))
    if trace:
        trace = _install_ntff_shim()
    res = run_bass_kernel_spmd(nc, in_maps, core_ids=list(range(NCORES)),
                               trace=trace)
    _CACHE["last_result"] = res
    return np.concatenate([res.results[r]["out"] for r in range(NCORES)], axis=0)
